# revision 15
# baseline (speedup 1.0000x reference)
"""Adaptive-histogram-equalization (6x6 tiles, 256 bins) Trainium2 kernel.

For TILE=6 the reference op is provably the identity: each 6x6 tile has
npix = 36 pixels, so torchvision's step = (npix - hist[last_nonzero_bin])
// 255 is 0 for every tile (hist[last] >= 1 -> numerator <= 35 < 255), and
the reference keeps the original pixels whenever step == 0.  The optimal
kernel is therefore a pure memory copy, which matches the problem's
memory-bound target regime: the NEFF must read the full 48 MB input from
HBM and write the full 48 MB output back.

Sharding: the image is split into 8 equal flat chunks (padded to 6 MiB per
core); each NeuronCore copies its chunk with ONE direct DRAM->DRAM
dma_start on the sync (SP HWDGE) ring.  A flat 6 MiB copy normalizes to 96
descriptors of 64 KiB (outer dim a multiple of 16), which the HWDGE sprays
evenly over all 16 SDMA engines; they then run back-to-back at the per-core
HBM limit (~21 B/ns/engine, ~340 GB/s one-way per core).  Measured NEFF
exec: ~30.5 us (vs 139 us for a naive uneven-split copy; rel err 0).

Pitfalls baked into this layout (measured on hardware):
- Chunks must split into uniform 64 KiB descriptors with the outer count a
  multiple of 16; odd-sized DMAs produce descriptor sets that land almost
  entirely on ONE SDMA engine (4-5x slower).
- Never issue DMAs from both HWDGE engines (sync + scalar) in one Block:
  that crashed the device (NRT_EXEC_UNIT_UNRECOVERABLE).
"""

import numpy as np

import concourse.bass as bass
import concourse.mybir as mybir
from concourse.bass_utils import run_bass_kernel_spmd

H = 2046
W = 2046
C = 3
TOTAL = H * W * C                     # 12,558,348 int32 elements
N_CORES = 8
CHUNK = 1_572_864                     # 6 MiB of int32 per core (padded)
PAD_TOTAL = CHUNK * N_CORES           # 12,582,912

_NC_CACHE = {}
LAST_RESULT = None  # BassKernelResults of the most recent run (for test.py)
RUN_KWARGS = {}     # extra kwargs for run_bass_kernel_spmd (for test.py)
BUILD_OPTS = {}     # build overrides for benchmarking (empty = shipped config)


def _build(
    max_last_dim: int | None = None,
    wait: bool = True,
    n_dma: int = 1,
    mix_gpsimd: bool = False,
    no_drain: bool = False,
    lean: bool = False,
    delay_insts: int = 0,
) -> bass.Bass:
    if lean:
        nc = bass.Bass(enable_partition_id=False, monotonic_sem_count=0)
    else:
        nc = bass.Bass()
    x = nc.declare_dram_parameter("x", [CHUNK], mybir.dt.int32, isOutput=False)
    out = nc.declare_dram_parameter("out", [CHUNK], mybir.dt.int32, isOutput=True)
    per = CHUNK // n_dma

    with (
        nc.Block(no_gpsimd_drain=no_drain) as block,
        nc.semaphore("dma_sem") as dma_sem,
    ):

        def make_body(dma_ids):
            def body(eng: bass.BassEngine):
                for _ in range(delay_insts):
                    eng.wait_ge(dma_sem, 0)
                for i in dma_ids:
                    eng.dma_start(
                        out=out[per * i : per * (i + 1)],
                        in_=x[per * i : per * (i + 1)],
                        max_dma_last_dim=max_last_dim,
                    ).then_inc(dma_sem, 16)
                if wait:
                    eng.wait_ge(dma_sem, 16 * n_dma)

            return body

        if mix_gpsimd:
            block.sync(make_body([i for i in range(n_dma) if i % 2 == 0]))
            block.gpsimd(make_body([i for i in range(n_dma) if i % 2 == 1]))
        else:
            block.sync(make_body(list(range(n_dma))))

    return nc


def kernel(pic: np.ndarray) -> np.ndarray:
    global LAST_RESULT
    pic = np.ascontiguousarray(pic, dtype=np.int32)

    padded = np.empty(PAD_TOTAL, np.int32)
    padded[:TOTAL] = pic.reshape(-1)
    padded[TOTAL:] = 0
    shards = padded.reshape(N_CORES, CHUNK)

    key = tuple(sorted(BUILD_OPTS.items()))
    if key not in _NC_CACHE:
        _NC_CACHE[key] = _build(**BUILD_OPTS)
    nc = _NC_CACHE[key]

    in_maps = [{"x": shards[i]} for i in range(N_CORES)]
    res = run_bass_kernel_spmd(nc, in_maps, list(range(N_CORES)), **RUN_KWARGS)
    LAST_RESULT = res

    out_flat = np.concatenate([res.results[i]["out"] for i in range(N_CORES)])
    return out_flat[:TOTAL].reshape(H, W, C)
